# revision 1
# baseline (speedup 1.0000x reference)
"""DH-SFNN (dendritic two-layer spiking net + leaky readout) on 8 Trainium2
NeuronCores, pure batch data parallelism (16 batch rows per core).

Per-core structure:
  ph0: load x [4000,768] f32, cast bf16 (ACT), DMA-xbar transpose -> xT [768,4000]
  ph1: MM1 (bf16 weight-stationary) -> PSUM; per-(branch,b) dendrite IIR via
       tensor_tensor_scan directly on PSUM (state seeded with -K to fold the
       bias constant exactly); branch-sum + K via STT -> U1 (bf16)
  ph2: membrane scan, 250 steps:
         y[t] = M[t-1] * alpha        (GPSIMD TT)
         M[t] = y[t] + U''[t]         (DVE TT)
         U''[t+1] -= (M[t] >= 1)      (DVE STT, is_ge + reverse1 subtract)
       then bulk spike extraction S = (M >= 1) -> bf16
  ph3: MM2 on spikes -> d2 scan -> U2 (reuses buffers)
  ph4: membrane scan layer 2
  ph5: MMo -> readout integrator scan (seeded -bo) -> exp(mo + bo) (ACT) ->
       PE transpose of 125-col chunks -> row softmax -> masked time-sum via
       fp32 matmul with 0/1 mask lhsT -> out [1,320]
"""
import sys
sys.path.insert(0, "/opt/trn_rl_repo")

import numpy as np
import ml_dtypes

import concourse.bass as bass
from concourse import bacc, masks
import concourse.tile as tile
import concourse.mybir as mybir
from concourse.bass_utils import run_bass_kernel_spmd

F32 = mybir.dt.float32
BF16 = mybir.dt.bfloat16
AL = mybir.AluOpType
ACTF = mybir.ActivationFunctionType

# problem dims
B, T, IN, H, OUT, BR = 128, 250, 700, 512, 20, 4
NCORES = 8
BL = B // NCORES          # 16 batch rows per core
BT = BL * T               # 4000
INP = 768                 # padded input dim (6*128; xbar needs cols%128)
NKC1 = 6                  # k-chunks for MM1
NB = BL // 2              # 8 b-pair column blocks of 500
NCB = 500                 # columns per block (2 b * 250 t)
HH = 4                    # h chunks (512 = 4*128)

_NC_CACHE = None


def _build_nc():
    nc = bacc.Bacc(None, target_bir_lowering=False)

    xin = nc.dram_tensor("xin", [BT, INP], F32, kind="ExternalInput")
    w1t = nc.dram_tensor("w1t", [INP, H * BR], BF16, kind="ExternalInput")
    w2t = nc.dram_tensor("w2t", [H, H * BR], BF16, kind="ExternalInput")
    wot = nc.dram_tensor("wot", [H, OUT], BF16, kind="ExternalInput")
    a1t = nc.dram_tensor("a1t", [128, HH, BL], F32, kind="ExternalInput")
    a2t = nc.dram_tensor("a2t", [128, HH, BL], F32, kind="ExternalInput")
    k1t = nc.dram_tensor("k1t", [128, HH], F32, kind="ExternalInput")
    k2t = nc.dram_tensor("k2t", [128, HH], F32, kind="ExternalInput")
    bc1 = nc.dram_tensor("bc1", [128, HH * BR], F32, kind="ExternalInput")
    nk1 = nc.dram_tensor("nk1", [128, HH * BR], F32, kind="ExternalInput")
    bc2 = nc.dram_tensor("bc2", [128, HH * BR], F32, kind="ExternalInput")
    nk2 = nc.dram_tensor("nk2", [128, HH * BR], F32, kind="ExternalInput")
    aoc = nc.dram_tensor("aoc", [OUT, 1], F32, kind="ExternalInput")
    nko = nc.dram_tensor("nko", [OUT, 1], F32, kind="ExternalInput")
    kot = nc.dram_tensor("kot", [OUT, 1], F32, kind="ExternalInput")
    tmask = nc.dram_tensor("tmask", [125, 2], F32, kind="ExternalInput")
    out_d = nc.dram_tensor("out", [1, BL * OUT], F32, kind="ExternalOutput")

    with tile.TileContext(nc) as tc:
        with (
            tc.tile_pool(name="const", bufs=1) as cpool,
            tc.tile_pool(name="U", bufs=1) as upool,
            tc.tile_pool(name="S", bufs=1) as spool,
            tc.tile_pool(name="dsl", bufs=2) as dpool,
            tc.tile_pool(name="sm", bufs=2) as smpool,
            tc.tile_pool(name="ps", bufs=3, space=bass.MemorySpace.PSUM) as ps,
            tc.tile_pool(name="pso", bufs=2, space=bass.MemorySpace.PSUM) as pso,
            tc.tile_pool(name="psf", bufs=1, space=bass.MemorySpace.PSUM) as psf,
        ):
            # ---- constants in SBUF
            w1s = []
            for k in range(NKC1):
                tl = cpool.tile([128, H * BR], BF16, tag=f"w1_{k}")
                nc.sync.dma_start(tl[:], w1t[k * 128:(k + 1) * 128, :])
                w1s.append(tl)
            w2s = []
            for k in range(HH):
                tl = cpool.tile([128, H * BR], BF16, tag=f"w2_{k}")
                nc.sync.dma_start(tl[:], w2t[k * 128:(k + 1) * 128, :])
                w2s.append(tl)
            wos = []
            for k in range(HH):
                tl = cpool.tile([128, OUT], BF16, tag=f"wo_{k}")
                nc.sync.dma_start(tl[:], wot[k * 128:(k + 1) * 128, :])
                wos.append(tl)
            a1 = cpool.tile([128, HH, BL], F32, tag="a1")
            a2 = cpool.tile([128, HH, BL], F32, tag="a2")
            k1 = cpool.tile([128, HH], F32, tag="k1")
            k2 = cpool.tile([128, HH], F32, tag="k2")
            bc1s = cpool.tile([128, HH * BR], F32, tag="bc1")
            nk1s = cpool.tile([128, HH * BR], F32, tag="nk1")
            bc2s = cpool.tile([128, HH * BR], F32, tag="bc2")
            nk2s = cpool.tile([128, HH * BR], F32, tag="nk2")
            aocs = cpool.tile([OUT, 1], F32, tag="aoc")
            nkos = cpool.tile([OUT, 1], F32, tag="nko")
            ko = cpool.tile([OUT, 1], F32, tag="ko")
            tm = cpool.tile([125, 2], F32, tag="tmask")
            for dst, src in ((a1, a1t), (a2, a2t), (k1, k1t), (k2, k2t),
                             (bc1s, bc1), (nk1s, nk1), (bc2s, bc2),
                             (nk2s, nk2), (aocs, aoc), (nkos, nko),
                             (ko, kot), (tm, tmask)):
                nc.sync.dma_start(dst[:], src[:])
            ident = cpool.tile([128, 128], F32, tag="ident")
            masks.make_identity(nc, ident[:])

            U = upool.tile([128, HH, BL, T], BF16, tag="U")
            S = spool.tile([128, HH, BL, T], BF16, tag="S")

            # ---- generic layer pipeline: MM + seeded d-scan + br-sum -> U
            def layer_mm_dscan(w_tiles, kws, rhs_fn, bcs, nks, ktile):
                for hh in range(HH):
                    for nb in range(NB):
                        dsl = []
                        for br in range(BR):
                            g = br * HH + hh
                            j = hh * BR + br
                            acc = ps.tile([128, NCB], F32, tag="mmps",
                                          name="mmps")
                            nk = len(w_tiles)
                            for k in range(nk):
                                nc.tensor.matmul(
                                    acc[:],
                                    w_tiles[k][:kws[k], g * 128:(g + 1) * 128],
                                    rhs_fn(k, nb),
                                    start=(k == 0), stop=(k == nk - 1))
                            dt_ = dpool.tile([128, NCB], F32, tag=f"d_{br}",
                                             name=f"d_{br}")
                            for bl in range(2):
                                nc.vector.tensor_tensor_scan(
                                    dt_[:, bl * T:(bl + 1) * T],
                                    bcs[:, j:j + 1].broadcast_to([128, T]),
                                    acc[:, bl * T:(bl + 1) * T],
                                    nks[:, j:j + 1],
                                    op0=AL.mult, op1=AL.add)
                            dsl.append(dt_)
                        t01 = dpool.tile([128, NCB], F32, tag="t01", name="t01")
                        nc.gpsimd.tensor_add(t01[:], dsl[0][:], dsl[1][:])
                        nc.gpsimd.tensor_add(t01[:], t01[:], dsl[2][:])
                        uview = U[:, hh, 2 * nb:2 * nb + 2, :].rearrange(
                            "p a b -> p (a b)")
                        nc.vector.scalar_tensor_tensor(
                            uview, t01[:], ktile[:, hh:hh + 1], dsl[3][:],
                            op0=AL.add, op1=AL.add)

            # ---- membrane scan v2 (fused spike writeback, M trajectory)
            def mscan(A, M):
                for t in range(T):
                    if t == 0:
                        nc.vector.tensor_copy(M[:, :, :, 0], U[:, :, :, 0])
                    else:
                        y = dpool.tile([128, HH, BL], F32, tag="msy", name="msy")
                        nc.gpsimd.tensor_mul(y[:], M[:, :, :, t - 1], A[:])
                        nc.vector.tensor_add(M[:, :, :, t], y[:], U[:, :, :, t])
                    if t < T - 1:
                        inst = nc.vector.scalar_tensor_tensor(
                            U[:, :, :, t + 1], M[:, :, :, t], 1.0,
                            U[:, :, :, t + 1], op0=AL.is_ge, op1=AL.subtract)
                        inst.ins.reverse1 = True
                # bulk spike extraction (per hh so MM2 can chase)
                for hh in range(HH):
                    nc.vector.tensor_scalar(
                        S[:, hh], M[:, hh], 1.0, None, op0=AL.is_ge)

            # ======== phase 0 + 1 (xT alive only here) ========
            with (
                tc.tile_pool(name="xT", bufs=1) as xpool,
                tc.tile_pool(name="stage", bufs=2) as stage,
            ):
                xT = []
                for k in range(NKC1):
                    xT.append(xpool.tile([128, BT], BF16, tag=f"xT_{k}",
                                         name=f"xT_{k}"))
                nrt = [(i, 128 if i < 31 else BT - 31 * 128) for i in range(32)]
                for i, rows in nrt:
                    xs = stage.tile([128, INP], F32, tag="xs", name="xs")
                    nc.sync.dma_start(xs[:rows, :], xin[i * 128:i * 128 + rows, :])
                    xb = stage.tile([128, INP], BF16, tag="xb", name="xb")
                    nc.scalar.copy(xb[:rows, :], xs[:rows, :])
                    for k in range(NKC1):
                        eng = nc.sync if (k % 2 == 0) else nc.scalar
                        eng.dma_start_transpose(
                            xT[k][:, i * 128:i * 128 + rows],
                            xb[:rows, k * 128:(k + 1) * 128])

                kws1 = [128] * 6
                layer_mm_dscan(
                    w1s, kws1,
                    lambda k, nb: xT[k][:, nb * NCB:(nb + 1) * NCB],
                    bc1s, nk1s, k1)

            # ======== phases 2-4 (M alive only here) ========
            with tc.tile_pool(name="M", bufs=1) as mpool:
                M = mpool.tile([128, HH, BL, T], F32, tag="M")
                mscan(a1, M)

                kws2 = [128] * 4
                layer_mm_dscan(
                    w2s, kws2,
                    lambda k, nb: S[:, k, 2 * nb:2 * nb + 2, :].rearrange(
                        "p a b -> p (a b)"),
                    bc2s, nk2s, k2)

                mscan(a2, M)

            # ======== phase 5: readout + softmax + masked time sum ========
            acc_f = psf.tile([1, BL * OUT], F32, tag="accf")
            for nb in range(NB):
                po = pso.tile([OUT, NCB], F32, tag="mops", name="mops")
                for k in range(HH):
                    nc.tensor.matmul(
                        po[:], wos[k][:],
                        S[:, k, 2 * nb:2 * nb + 2, :].rearrange(
                            "p a b -> p (a b)"),
                        start=(k == 0), stop=(k == HH - 1))
                mo = smpool.tile([OUT, NCB], F32, tag="mo", name="mo")
                for bl in range(2):
                    nc.vector.tensor_tensor_scan(
                        mo[:, bl * T:(bl + 1) * T],
                        aocs[:].broadcast_to([OUT, T]),
                        po[:, bl * T:(bl + 1) * T],
                        nkos[:],
                        op0=AL.mult, op1=AL.add)
                ex = smpool.tile([OUT, NCB], F32, tag="ex", name="ex")
                nc.scalar.activation(ex[:], mo[:], ACTF.Exp,
                                     bias=ko[:], scale=1.0)
                for bl in range(2):
                    b = 2 * nb + bl
                    for half in range(2):
                        ptr = pso.tile([125, OUT], F32, tag="trps", name="trps")
                        nc.tensor.transpose(
                            ptr[:], ex[:, bl * T + half * 125:
                                        bl * T + (half + 1) * 125],
                            ident[:OUT, :OUT])
                        et = smpool.tile([125, OUT], F32, tag="et", name="et")
                        nc.scalar.copy(et[:], ptr[:])
                        sm_sum = smpool.tile([125, 1], F32, tag="sms", name="sms")
                        nc.vector.reduce_sum(sm_sum[:], et[:],
                                             axis=mybir.AxisListType.X)
                        rc = smpool.tile([125, 1], F32, tag="rc", name="rc")
                        nc.vector.reciprocal(rc[:], sm_sum[:])
                        pr = smpool.tile([125, OUT], F32, tag="pr", name="pr")
                        nc.vector.tensor_scalar(
                            pr[:], et[:], rc[:], None, op0=AL.mult)
                        nc.tensor.matmul(
                            acc_f[:, b * OUT:(b + 1) * OUT],
                            tm[:, half:half + 1], pr[:],
                            start=(half == 0), stop=(half == 1))
            fin = smpool.tile([1, BL * OUT], F32, tag="fin", name="fin")
            nc.scalar.copy(fin[:], acc_f[:])
            nc.sync.dma_start(out_d[:], fin[:])

    nc.compile()
    return nc


def _sigmoid(x):
    return 1.0 / (1.0 + np.exp(-x.astype(np.float64)))


def _host_prep(inputs):
    f32 = np.float32
    x = np.asarray(inputs["x"], f32)
    W1 = np.asarray(inputs["W1"], f32); b1 = np.asarray(inputs["b1"], f32)
    W2 = np.asarray(inputs["W2"], f32); b2 = np.asarray(inputs["b2"], f32)
    Wo = np.asarray(inputs["Wo"], f32); bo = np.asarray(inputs["bo"], f32)
    mask1 = np.asarray(inputs["mask1"], f32); mask2 = np.asarray(inputs["mask2"], f32)
    beta1 = _sigmoid(np.asarray(inputs["tau_n1"], f32)).astype(f32)   # [H,BR]
    alpha1 = _sigmoid(np.asarray(inputs["tau_m1"], f32)).astype(f32)  # [H]
    beta2 = _sigmoid(np.asarray(inputs["tau_n2"], f32)).astype(f32)
    alpha2 = _sigmoid(np.asarray(inputs["tau_m2"], f32)).astype(f32)
    alpha_o = _sigmoid(np.asarray(inputs["tau_mo"], f32)).astype(f32)  # [OUT]

    lam1 = ((1.0 - beta1) * (1.0 - alpha1)[:, None]).astype(f32)      # [H,BR]
    lam2 = ((1.0 - beta2) * (1.0 - alpha2)[:, None]).astype(f32)

    Wm1 = (W1 * mask1).reshape(H, BR, IN)
    W1p = (lam1[:, :, None] * Wm1).transpose(1, 0, 2).reshape(BR * H, IN)
    W1T = np.zeros((INP, H * BR), f32)
    W1T[:IN, :] = W1p.T
    W1T = W1T.astype(ml_dtypes.bfloat16)

    Wm2 = (W2 * mask2).reshape(H, BR, H)
    W2p = (lam2[:, :, None] * Wm2).transpose(1, 0, 2).reshape(BR * H, H)
    W2T = np.ascontiguousarray(W2p.T).astype(ml_dtypes.bfloat16)

    Wop = ((1.0 - alpha_o)[:, None] * Wo)
    WoT = np.ascontiguousarray(Wop.T).astype(ml_dtypes.bfloat16)      # [512,20]

    b1r = b1.reshape(H, BR); b2r = b2.reshape(H, BR)
    K1br = ((1.0 - alpha1)[:, None] * b1r).astype(f32)                # [H,BR]
    K2br = ((1.0 - alpha2)[:, None] * b2r).astype(f32)
    K1 = K1br.sum(1)                                                  # [H]
    K2 = K2br.sum(1)
    Ko = bo.astype(f32)                                               # [OUT]

    a1t = np.broadcast_to(
        alpha1.reshape(HH, 128).T[:, :, None], (128, HH, BL)).astype(f32).copy()
    a2t = np.broadcast_to(
        alpha2.reshape(HH, 128).T[:, :, None], (128, HH, BL)).astype(f32).copy()
    k1t = np.ascontiguousarray(K1.reshape(HH, 128).T).astype(f32)
    k2t = np.ascontiguousarray(K2.reshape(HH, 128).T).astype(f32)

    bc1 = np.zeros((128, HH * BR), f32)
    nk1 = np.zeros((128, HH * BR), f32)
    bc2 = np.zeros((128, HH * BR), f32)
    nk2 = np.zeros((128, HH * BR), f32)
    for hh in range(HH):
        for br in range(BR):
            j = hh * BR + br
            bc1[:, j] = beta1[hh * 128:(hh + 1) * 128, br]
            nk1[:, j] = -K1br[hh * 128:(hh + 1) * 128, br]
            bc2[:, j] = beta2[hh * 128:(hh + 1) * 128, br]
            nk2[:, j] = -K2br[hh * 128:(hh + 1) * 128, br]
    aoc = alpha_o.reshape(OUT, 1).astype(f32)
    nko = (-Ko).reshape(OUT, 1).astype(f32)
    kot = Ko.reshape(OUT, 1).astype(f32)
    tmask = np.zeros((125, 2), f32)
    tmask[11:, 0] = 1.0
    tmask[:, 1] = 1.0

    shared = dict(w1t=W1T, w2t=W2T, wot=WoT, a1t=a1t, a2t=a2t, k1t=k1t,
                  k2t=k2t, bc1=bc1, nk1=nk1, bc2=bc2, nk2=nk2, aoc=aoc,
                  nko=nko, kot=kot, tmask=tmask)
    xs = []
    for c in range(NCORES):
        xc = x[c * BL:(c + 1) * BL].reshape(BT, IN)
        xp = np.zeros((BT, INP), f32)
        xp[:, :IN] = xc
        xs.append(xp)
    return shared, xs


def kernel(**inputs):
    global _NC_CACHE
    if _NC_CACHE is None:
        _NC_CACHE = _build_nc()
    nc = _NC_CACHE
    shared, xs = _host_prep(inputs)
    in_maps = [dict(shared, xin=xs[c]) for c in range(NCORES)]
    res = run_bass_kernel_spmd(nc, in_maps, core_ids=list(range(NCORES)))
    out = np.concatenate(
        [res.results[c]["out"].reshape(BL, OUT) for c in range(NCORES)], axis=0)
    return out.astype(np.float32)



# revision 5
# speedup vs baseline: 1.1246x; 1.1246x over previous
"""DH-SFNN (dendritic two-layer spiking net + leaky readout) on 8 Trainium2
NeuronCores, pure batch data parallelism (16 batch rows per core).

Per-core structure:
  ph0: load x [4000,768] f32, cast bf16 (ACT), DMA-xbar transpose -> xT [768,4000]
  ph1: MM1 (bf16 weight-stationary) -> PSUM; per-(branch,b) dendrite IIR via
       tensor_tensor_scan directly on PSUM (state seeded with -K to fold the
       bias constant exactly); branch-sum + K via STT -> U1 (bf16)
  ph2: membrane scan, 250 steps:
         y[t] = M[t-1] * alpha        (GPSIMD TT)
         M[t] = y[t] + U''[t]         (DVE TT)
         U''[t+1] -= (M[t] >= 1)      (DVE STT, is_ge + reverse1 subtract)
       then bulk spike extraction S = (M >= 1) -> bf16
  ph3: MM2 on spikes -> d2 scan -> U2 (reuses buffers)
  ph4: membrane scan layer 2
  ph5: MMo -> readout integrator scan (seeded -bo) -> exp(mo + bo) (ACT) ->
       PE transpose of 125-col chunks -> row softmax -> masked time-sum via
       fp32 matmul with 0/1 mask lhsT -> out [1,320]
"""
import sys
sys.path.insert(0, "/opt/trn_rl_repo")

import numpy as np
import ml_dtypes

import concourse.bass as bass
from concourse import bacc, masks
import concourse.tile as tile
import concourse.mybir as mybir
from concourse.bass_utils import run_bass_kernel_spmd

# ---- custom DVE op: z = a*m - (m >= 1), fusing the membrane decay-multiply
# with the soft-reset spike subtract into one Vector instruction.
from concourse.dve_ops import (DveOp, OPS, CUSTOM_DVE_SPECS, DveOpSpec,
                               _SUB_OPCODE_FOR_NAME, _CUSTOM_DVE_ROW_BASE)
from concourse.dve_spec import (Spec, Src0, Src1, One, Zero, select,
                                lower as _dve_lower)


def _register_dve(name, spec, subdim=False):
    if name in _SUB_OPCODE_FOR_NAME:
        return next(o for o in OPS if o.name == name)
    row = _CUSTOM_DVE_ROW_BASE + len(OPS)
    assert row < 0x20
    shas = {v: DveOpSpec(name=name, opcode=row, uops=_dve_lower(spec, ver=v),
                         rd1_en=True).sha(v) for v in ("v3", "v4")}
    op = DveOp(name, spec, subdim=subdim, uops_sha=shas)
    OPS.append(op)
    CUSTOM_DVE_SPECS[name] = spec
    _SUB_OPCODE_FOR_NAME[name] = row
    return op


MDECAY = _register_dve("MDECAY", Spec(
    body=Src0 * Src1 - select(Src0 >= One, One, Zero),
    reference=lambda in0, in1, s0, s1, imm2:
        in0 * in1 - (in0 >= 1.0).astype(np.float32),
))

F32 = mybir.dt.float32
BF16 = mybir.dt.bfloat16
AL = mybir.AluOpType
ACTF = mybir.ActivationFunctionType

# problem dims
B, T, IN, H, OUT, BR = 128, 250, 700, 512, 20, 4
NCORES = 8
BL = B // NCORES          # 16 batch rows per core
BT = BL * T               # 4000
INP = 768                 # padded input dim (6*128; xbar needs cols%128)
NKC1 = 6                  # k-chunks for MM1
NB = BL // 2              # 8 b-pair column blocks of 500
NCB = 500                 # columns per block (2 b * 250 t)
HH = 4                    # h chunks (512 = 4*128)

_NC_CACHE = None


def _build_nc():
    nc = bacc.Bacc(None, target_bir_lowering=False)

    xin = nc.dram_tensor("xin", [BT, INP], F32, kind="ExternalInput")
    w1t = nc.dram_tensor("w1t", [INP, H * BR], BF16, kind="ExternalInput")
    w2t = nc.dram_tensor("w2t", [H, H * BR], BF16, kind="ExternalInput")
    wot = nc.dram_tensor("wot", [H, OUT], BF16, kind="ExternalInput")
    a1t = nc.dram_tensor("a1t", [128, HH, BL], F32, kind="ExternalInput")
    a2t = nc.dram_tensor("a2t", [128, HH, BL], F32, kind="ExternalInput")
    k1t = nc.dram_tensor("k1t", [128, HH], F32, kind="ExternalInput")
    k2t = nc.dram_tensor("k2t", [128, HH], F32, kind="ExternalInput")
    bc1 = nc.dram_tensor("bc1", [128, HH * BR], F32, kind="ExternalInput")
    nk1 = nc.dram_tensor("nk1", [128, HH * BR], F32, kind="ExternalInput")
    bc2 = nc.dram_tensor("bc2", [128, HH * BR], F32, kind="ExternalInput")
    nk2 = nc.dram_tensor("nk2", [128, HH * BR], F32, kind="ExternalInput")
    aoc = nc.dram_tensor("aoc", [OUT, 1], F32, kind="ExternalInput")
    nko = nc.dram_tensor("nko", [OUT, 1], F32, kind="ExternalInput")
    kot = nc.dram_tensor("kot", [OUT, 1], F32, kind="ExternalInput")
    tmask = nc.dram_tensor("tmask", [125, 2], F32, kind="ExternalInput")
    out_d = nc.dram_tensor("out", [1, BL * OUT], F32, kind="ExternalOutput")

    with tile.TileContext(nc) as tc:
        with (
            tc.tile_pool(name="const", bufs=1) as cpool,
            tc.tile_pool(name="U", bufs=1) as upool,
            tc.tile_pool(name="S", bufs=1) as spool,
            tc.tile_pool(name="dsl", bufs=2) as dpool,
            tc.tile_pool(name="sm", bufs=2) as smpool,
            tc.tile_pool(name="ps", bufs=3, space=bass.MemorySpace.PSUM) as ps,
            tc.tile_pool(name="pso", bufs=2, space=bass.MemorySpace.PSUM) as pso,
            tc.tile_pool(name="psf", bufs=1, space=bass.MemorySpace.PSUM) as psf,
        ):
            # ---- constants in SBUF
            w1s = []
            for k in range(NKC1):
                tl = cpool.tile([128, H * BR], BF16, tag=f"w1_{k}")
                nc.sync.dma_start(tl[:], w1t[k * 128:(k + 1) * 128, :])
                w1s.append(tl)
            w2s = []
            for k in range(HH):
                tl = cpool.tile([128, H * BR], BF16, tag=f"w2_{k}")
                nc.sync.dma_start(tl[:], w2t[k * 128:(k + 1) * 128, :])
                w2s.append(tl)
            wos = []
            for k in range(HH):
                tl = cpool.tile([128, OUT], BF16, tag=f"wo_{k}")
                nc.sync.dma_start(tl[:], wot[k * 128:(k + 1) * 128, :])
                wos.append(tl)
            a1 = cpool.tile([128, HH, BL], F32, tag="a1")
            a2 = cpool.tile([128, HH, BL], F32, tag="a2")
            k1 = cpool.tile([128, HH], F32, tag="k1")
            k2 = cpool.tile([128, HH], F32, tag="k2")
            bc1s = cpool.tile([128, HH * BR], F32, tag="bc1")
            nk1s = cpool.tile([128, HH * BR], F32, tag="nk1")
            bc2s = cpool.tile([128, HH * BR], F32, tag="bc2")
            nk2s = cpool.tile([128, HH * BR], F32, tag="nk2")
            aocs = cpool.tile([OUT, 1], F32, tag="aoc")
            nkos = cpool.tile([OUT, 1], F32, tag="nko")
            ko = cpool.tile([OUT, 1], F32, tag="ko")
            tm = cpool.tile([125, 2], F32, tag="tmask")
            for dst, src in ((a1, a1t), (a2, a2t), (k1, k1t), (k2, k2t),
                             (bc1s, bc1), (nk1s, nk1), (bc2s, bc2),
                             (nk2s, nk2), (aocs, aoc), (nkos, nko),
                             (ko, kot), (tm, tmask)):
                nc.sync.dma_start(dst[:], src[:])
            ident = cpool.tile([128, 128], F32, tag="ident")
            masks.make_identity(nc, ident[:])

            U = upool.tile([128, HH, BL, T], BF16, tag="U")
            S = spool.tile([128, HH, BL, T], BF16, tag="S")

            # ---- generic layer pipeline: MM + seeded d-scan + br-sum -> U
            def layer_mm_dscan(w_tiles, kws, rhs_fn, bcs, nks, ktile):
                for hh in range(HH):
                    for nb in range(NB):
                        dsl = []
                        for br in range(BR):
                            g = br * HH + hh
                            j = hh * BR + br
                            acc = ps.tile([128, NCB], F32, tag="mmps",
                                          name="mmps")
                            nk = len(w_tiles)
                            for k in range(nk):
                                nc.tensor.matmul(
                                    acc[:],
                                    w_tiles[k][:kws[k], g * 128:(g + 1) * 128],
                                    rhs_fn(k, nb),
                                    start=(k == 0), stop=(k == nk - 1))
                            dt_ = dpool.tile([128, NCB], F32, tag=f"d_{br}",
                                             name=f"d_{br}")
                            for bl in range(2):
                                nc.vector.tensor_tensor_scan(
                                    dt_[:, bl * T:(bl + 1) * T],
                                    bcs[:, j:j + 1].broadcast_to([128, T]),
                                    acc[:, bl * T:(bl + 1) * T],
                                    nks[:, j:j + 1],
                                    op0=AL.mult, op1=AL.add)
                            dsl.append(dt_)
                        t01 = dpool.tile([128, NCB], F32, tag="t01", name="t01")
                        nc.gpsimd.tensor_add(t01[:], dsl[0][:], dsl[1][:])
                        nc.gpsimd.tensor_add(t01[:], t01[:], dsl[2][:])
                        uview = U[:, hh, 2 * nb:2 * nb + 2, :].rearrange(
                            "p a b -> p (a b)")
                        nc.vector.scalar_tensor_tensor(
                            uview, t01[:], ktile[:, hh:hh + 1], dsl[3][:],
                            op0=AL.add, op1=AL.add)

            # ---- membrane scan v3: 2 fused DVE ops per step, U read-only
            #   z[t] = a*M[t-1] - (M[t-1] >= 1)     (custom MDECAY)
            #   M[t] = z[t] + U[t]                  (stock add)
            def mscan(A, M):
                for t in range(T):
                    if t == 0:
                        nc.vector.tensor_copy(M[:, :, :, 0], U[:, :, :, 0])
                    else:
                        z = dpool.tile([128, HH, BL], F32, tag="msz",
                                       name="msz")
                        nc.vector._custom_dve(MDECAY, out=z[:],
                                              in0=M[:, :, :, t - 1], in1=A[:])
                        nc.vector.tensor_add(M[:, :, :, t], z[:],
                                             U[:, :, :, t])
                # bulk spike extraction (per hh so MM2 can chase)
                for hh in range(HH):
                    nc.vector.tensor_scalar(
                        S[:, hh], M[:, hh], 1.0, None, op0=AL.is_ge)

            # ======== phase 0 + 1 (xT alive only here) ========
            with (
                tc.tile_pool(name="xT", bufs=1) as xpool,
                tc.tile_pool(name="stage", bufs=2) as stage,
            ):
                xT = []
                for k in range(NKC1):
                    xT.append(xpool.tile([128, BT], BF16, tag=f"xT_{k}",
                                         name=f"xT_{k}"))
                nrt = [(i, 128 if i < 31 else BT - 31 * 128) for i in range(32)]
                for i, rows in nrt:
                    xs = stage.tile([128, INP], F32, tag="xs", name="xs")
                    nc.sync.dma_start(xs[:rows, :], xin[i * 128:i * 128 + rows, :])
                    xb = stage.tile([128, INP], BF16, tag="xb", name="xb")
                    nc.scalar.copy(xb[:rows, :], xs[:rows, :])
                    for k in range(NKC1):
                        eng = nc.sync if (k % 2 == 0) else nc.scalar
                        eng.dma_start_transpose(
                            xT[k][:, i * 128:i * 128 + rows],
                            xb[:rows, k * 128:(k + 1) * 128])

                kws1 = [128] * 6
                layer_mm_dscan(
                    w1s, kws1,
                    lambda k, nb: xT[k][:, nb * NCB:(nb + 1) * NCB],
                    bc1s, nk1s, k1)

            # ======== phases 2-4 (M alive only here) ========
            with tc.tile_pool(name="M", bufs=1) as mpool:
                M = mpool.tile([128, HH, BL, T], F32, tag="M")
                mscan(a1, M)

                kws2 = [128] * 4
                layer_mm_dscan(
                    w2s, kws2,
                    lambda k, nb: S[:, k, 2 * nb:2 * nb + 2, :].rearrange(
                        "p a b -> p (a b)"),
                    bc2s, nk2s, k2)

                mscan(a2, M)

            # ======== phase 5: readout + softmax + masked time sum ========
            acc_f = psf.tile([1, BL * OUT], F32, tag="accf")
            for nb in range(NB):
                po = pso.tile([OUT, NCB], F32, tag="mops", name="mops")
                for k in range(HH):
                    nc.tensor.matmul(
                        po[:], wos[k][:],
                        S[:, k, 2 * nb:2 * nb + 2, :].rearrange(
                            "p a b -> p (a b)"),
                        start=(k == 0), stop=(k == HH - 1))
                mo = smpool.tile([OUT, NCB], F32, tag="mo", name="mo")
                for bl in range(2):
                    nc.vector.tensor_tensor_scan(
                        mo[:, bl * T:(bl + 1) * T],
                        aocs[:].broadcast_to([OUT, T]),
                        po[:, bl * T:(bl + 1) * T],
                        nkos[:],
                        op0=AL.mult, op1=AL.add)
                ex = smpool.tile([OUT, NCB], F32, tag="ex", name="ex")
                nc.scalar.activation(ex[:], mo[:], ACTF.Exp,
                                     bias=ko[:], scale=1.0)
                for bl in range(2):
                    b = 2 * nb + bl
                    for half in range(2):
                        ptr = pso.tile([125, OUT], F32, tag="trps", name="trps")
                        nc.tensor.transpose(
                            ptr[:], ex[:, bl * T + half * 125:
                                        bl * T + (half + 1) * 125],
                            ident[:OUT, :OUT])
                        et = smpool.tile([125, OUT], F32, tag="et", name="et")
                        nc.scalar.copy(et[:], ptr[:])
                        sm_sum = smpool.tile([125, 1], F32, tag="sms", name="sms")
                        nc.vector.reduce_sum(sm_sum[:], et[:],
                                             axis=mybir.AxisListType.X)
                        rc = smpool.tile([125, 1], F32, tag="rc", name="rc")
                        nc.vector.reciprocal(rc[:], sm_sum[:])
                        pr = smpool.tile([125, OUT], F32, tag="pr", name="pr")
                        nc.vector.tensor_scalar(
                            pr[:], et[:], rc[:], None, op0=AL.mult)
                        nc.tensor.matmul(
                            acc_f[:, b * OUT:(b + 1) * OUT],
                            tm[:, half:half + 1], pr[:],
                            start=(half == 0), stop=(half == 1))
            fin = smpool.tile([1, BL * OUT], F32, tag="fin", name="fin")
            nc.scalar.copy(fin[:], acc_f[:])
            nc.sync.dma_start(out_d[:], fin[:])

    nc.compile()
    return nc


def _sigmoid(x):
    return 1.0 / (1.0 + np.exp(-x.astype(np.float64)))


def _host_prep(inputs):
    f32 = np.float32
    x = np.asarray(inputs["x"], f32)
    W1 = np.asarray(inputs["W1"], f32); b1 = np.asarray(inputs["b1"], f32)
    W2 = np.asarray(inputs["W2"], f32); b2 = np.asarray(inputs["b2"], f32)
    Wo = np.asarray(inputs["Wo"], f32); bo = np.asarray(inputs["bo"], f32)
    mask1 = np.asarray(inputs["mask1"], f32); mask2 = np.asarray(inputs["mask2"], f32)
    beta1 = _sigmoid(np.asarray(inputs["tau_n1"], f32)).astype(f32)   # [H,BR]
    alpha1 = _sigmoid(np.asarray(inputs["tau_m1"], f32)).astype(f32)  # [H]
    beta2 = _sigmoid(np.asarray(inputs["tau_n2"], f32)).astype(f32)
    alpha2 = _sigmoid(np.asarray(inputs["tau_m2"], f32)).astype(f32)
    alpha_o = _sigmoid(np.asarray(inputs["tau_mo"], f32)).astype(f32)  # [OUT]

    lam1 = ((1.0 - beta1) * (1.0 - alpha1)[:, None]).astype(f32)      # [H,BR]
    lam2 = ((1.0 - beta2) * (1.0 - alpha2)[:, None]).astype(f32)

    Wm1 = (W1 * mask1).reshape(H, BR, IN)
    W1p = (lam1[:, :, None] * Wm1).transpose(1, 0, 2).reshape(BR * H, IN)
    W1T = np.zeros((INP, H * BR), f32)
    W1T[:IN, :] = W1p.T
    W1T = W1T.astype(ml_dtypes.bfloat16)

    Wm2 = (W2 * mask2).reshape(H, BR, H)
    W2p = (lam2[:, :, None] * Wm2).transpose(1, 0, 2).reshape(BR * H, H)
    W2T = np.ascontiguousarray(W2p.T).astype(ml_dtypes.bfloat16)

    Wop = ((1.0 - alpha_o)[:, None] * Wo)
    WoT = np.ascontiguousarray(Wop.T).astype(ml_dtypes.bfloat16)      # [512,20]

    b1r = b1.reshape(H, BR); b2r = b2.reshape(H, BR)
    K1br = ((1.0 - alpha1)[:, None] * b1r).astype(f32)                # [H,BR]
    K2br = ((1.0 - alpha2)[:, None] * b2r).astype(f32)
    K1 = K1br.sum(1)                                                  # [H]
    K2 = K2br.sum(1)
    Ko = bo.astype(f32)                                               # [OUT]

    a1t = np.broadcast_to(
        alpha1.reshape(HH, 128).T[:, :, None], (128, HH, BL)).astype(f32).copy()
    a2t = np.broadcast_to(
        alpha2.reshape(HH, 128).T[:, :, None], (128, HH, BL)).astype(f32).copy()
    k1t = np.ascontiguousarray(K1.reshape(HH, 128).T).astype(f32)
    k2t = np.ascontiguousarray(K2.reshape(HH, 128).T).astype(f32)

    bc1 = np.zeros((128, HH * BR), f32)
    nk1 = np.zeros((128, HH * BR), f32)
    bc2 = np.zeros((128, HH * BR), f32)
    nk2 = np.zeros((128, HH * BR), f32)
    for hh in range(HH):
        for br in range(BR):
            j = hh * BR + br
            bc1[:, j] = beta1[hh * 128:(hh + 1) * 128, br]
            nk1[:, j] = -K1br[hh * 128:(hh + 1) * 128, br]
            bc2[:, j] = beta2[hh * 128:(hh + 1) * 128, br]
            nk2[:, j] = -K2br[hh * 128:(hh + 1) * 128, br]
    aoc = alpha_o.reshape(OUT, 1).astype(f32)
    nko = (-Ko).reshape(OUT, 1).astype(f32)
    kot = Ko.reshape(OUT, 1).astype(f32)
    tmask = np.zeros((125, 2), f32)
    tmask[11:, 0] = 1.0
    tmask[:, 1] = 1.0

    shared = dict(w1t=W1T, w2t=W2T, wot=WoT, a1t=a1t, a2t=a2t, k1t=k1t,
                  k2t=k2t, bc1=bc1, nk1=nk1, bc2=bc2, nk2=nk2, aoc=aoc,
                  nko=nko, kot=kot, tmask=tmask)
    xs = []
    for c in range(NCORES):
        xc = x[c * BL:(c + 1) * BL].reshape(BT, IN)
        xp = np.zeros((BT, INP), f32)
        xp[:, :IN] = xc
        xs.append(xp)
    return shared, xs


def kernel(**inputs):
    global _NC_CACHE
    if _NC_CACHE is None:
        _NC_CACHE = _build_nc()
    nc = _NC_CACHE
    shared, xs = _host_prep(inputs)
    in_maps = [dict(shared, xin=xs[c]) for c in range(NCORES)]
    res = run_bass_kernel_spmd(nc, in_maps, core_ids=list(range(NCORES)))
    out = np.concatenate(
        [res.results[c]["out"].reshape(BL, OUT) for c in range(NCORES)], axis=0)
    return out.astype(np.float32)

